# revision 6
# baseline (speedup 1.0000x reference)
"""Trainium2 Bass kernel for a 16-head self-attention encoder block.

Problem (fp32):
    x: (4, 2048, 1024);  Wq/Wk/Wv/Wo: (1024, 1024);  b*: (1024,)
    q/k/v = x @ W*.T + b*   (reshaped to 16 heads x 64)
    out   = softmax(q k^T) v @ Wo.T + bo     (no scaling, no mask)

Sharding over 8 cores: (batch n in 0..3) x (head-group hg in 0..1, 8 heads
each).  Each core computes, for its batch element and its 8 heads:
    QT/KT = (x @ Wq_s.T).T  in [feat(512), seq(2048)] layout
    V     =  x @ Wv_s.T     in [seq(2048), feat(512)] layout (+ ones col)
    per head: esT = exp(K_h Q_h^T) in [k, q] layout (flash-style, no HBM
    round-trip); ctxT_h = V_aug_h^T @ esT gives both the unnormalised
    context and the softmax denominators (ones column) in one accumulation;
    normalise, then outT_part = Wo_s^T-slice @ ctxT.
Host side: out[n] = (outT_part[n,0] + outT_part[n,1]).T + (bv @ Wo.T + bo).
The bv/bo terms fold into a constant row because softmax rows sum to 1.

Matmuls run in float32r (TF32-like, full PE rate for moving dim >= 256).
"""
import os
import numpy as np

import concourse.bacc as bacc
import concourse.tile as tile
from concourse import mybir, bass_utils

F32 = mybir.dt.float32
F32R = mybir.dt.float32r
AF = mybir.ActivationFunctionType

N, L, H = 4, 2048, 1024
HH = H // 2          # per-core head dim (8 heads x 64)
P = 128
KC = H // P          # 8 contraction chunks for QKV projections
FB = HH // P         # 4 feature pblocks per projection
QC = L // 512        # 4 query chunks
KB = L // P          # 16 key blocks
HPC = 8              # heads per core
NCORES = 8

_CACHE: dict = {}
LAST_RESULTS = None


def _emit(nc, tc, io):
    from contextlib import ExitStack

    with ExitStack() as ctx:
        persist = ctx.enter_context(tc.tile_pool(name="persist", bufs=1))

        # --- long-lived weight / bias / constant tiles ---
        w_tiles = {}
        for wname in ("wq", "wk", "wv"):
            for kc in range(KC):
                t = persist.tile([P, HH], F32R, name=f"{wname}{kc}",
                                 tag=f"{wname}{kc}")
                nc.sync.dma_start(t[:], io[wname][kc * P:(kc + 1) * P, :].bitcast(F32R))
                w_tiles[(wname, kc)] = t
        bq_sb = persist.tile([P, FB], F32, name="bq_sb", tag="bq_sb")
        nc.sync.dma_start(bq_sb[:], io["bq"][:])
        bk_sb = persist.tile([P, FB], F32, name="bk_sb", tag="bk_sb")
        nc.sync.dma_start(bk_sb[:], io["bk"][:])
        ones8 = persist.tile([P, HPC], F32, name="ones8", tag="ones8")
        nc.vector.memset(ones8[:], 1.0)
        ones64f = persist.tile([1, 64], F32, name="ones64f", tag="ones64f")
        nc.vector.memset(ones64f[:], 1.0)
        ones64 = persist.tile([1, 64], F32R, name="ones64", tag="ones64")
        nc.scalar.copy(ones64[:], ones64f[:])

        qt = [persist.tile([P, L], F32R, name=f"qt{i}", tag=f"qt{i}")
              for i in range(FB)]
        kt = [persist.tile([P, L], F32R, name=f"kt{i}", tag=f"kt{i}")
              for i in range(FB)]
        vt = [persist.tile([P, HPC, 65], F32R, name=f"v{sb}", tag=f"v{sb}")
              for sb in range(KB)]

        # ================= phase 1: QKV projections =================
        with tc.tile_pool(name="xt", bufs=8) as xt_pool, \
             tc.tile_pool(name="ppj", bufs=4, space="PSUM") as ppj:
            for ch in range(2):            # column halves of x^T (seq dim)
                xts = []
                for kc in range(KC):
                    t = xt_pool.tile([P, L // 2], F32R, name=f"xt{ch}_{kc}",
                                     tag="xt")
                    nc.sync.dma_start(
                        t[:],
                        io["xT"][kc * P:(kc + 1) * P,
                                 ch * (L // 2):(ch + 1) * (L // 2)].bitcast(F32R))
                    xts.append(t)

                # QT / KT: [feat, seq] = W_s @ x^T
                for wname, dst, bias in (("wq", qt, bq_sb), ("wk", kt, bk_sb)):
                    for f in range(FB):
                        for c in range(2):   # 512-wide seq chunks in half
                            pj = ppj.tile([P, 512], F32, name="pj", tag="pj")
                            for kc in range(KC):
                                nc.tensor.matmul(
                                    pj[:],
                                    w_tiles[(wname, kc)][:, f * P:(f + 1) * P],
                                    xts[kc][:, c * 512:(c + 1) * 512],
                                    start=(kc == 0), stop=(kc == KC - 1))
                            col0 = ch * (L // 2) + c * 512
                            nc.scalar.add(dst[f][:, col0:col0 + 512], pj[:],
                                          bias[:, f:f + 1])
                # V: [seq, feat] = x @ Wv_s.T  (+ ones column per head)
                for s in range(KB // 2):
                    sb = ch * (KB // 2) + s
                    pj = ppj.tile([P, 512], F32, name="pj", tag="pj")
                    for kc in range(KC):
                        nc.tensor.matmul(
                            pj[:],
                            xts[kc][:, s * P:(s + 1) * P],
                            w_tiles[("wv", kc)][:],
                            start=(kc == 0), stop=(kc == KC - 1))
                    nc.vector.tensor_copy(
                        vt[sb][:, :, 0:64],
                        pj[:].rearrange("p (h s) -> p h s", h=HPC))
                    nc.vector.tensor_copy(vt[sb][:, :, 64], ones8[:])

        # ================= phase 2: attention + output projection =========
        with ExitStack() as ctx2:
            p2 = ctx2.enter_context(tc.tile_pool(name="p2", bufs=1))
            es_pool = ctx2.enter_context(tc.tile_pool(name="es", bufs=2))
            rd_pool = ctx2.enter_context(tc.tile_pool(name="rd", bufs=2))
            bcs_pool = ctx2.enter_context(tc.tile_pool(name="bcs", bufs=2))
            outst = ctx2.enter_context(tc.tile_pool(name="outst", bufs=2))
            sp_pool = ctx2.enter_context(
                tc.tile_pool(name="sp", bufs=1, space="PSUM"))
            acc_pool = ctx2.enter_context(
                tc.tile_pool(name="acc", bufs=2, space="PSUM"))
            bc_pool = ctx2.enter_context(
                tc.tile_pool(name="bc", bufs=1, space="PSUM"))

            wo = []
            for k2 in range(FB):
                t = p2.tile([P, H], F32R, name=f"wo{k2}", tag=f"wo{k2}")
                nc.sync.dma_start(
                    t[:], io["wo"][k2 * P:(k2 + 1) * P, :].bitcast(F32R))
                wo.append(t)
            ctx_pool = ctx2.enter_context(tc.tile_pool(name="cx", bufs=2))

            for c in range(QC):
                cx = [ctx_pool.tile([P, 512], F32R, name=f"cx{i}",
                                    tag=f"cx{i}") for i in range(FB)]
                for h in range(HPC):
                    hp, hr = h // 2, (h % 2) * 64
                    pc = acc_pool.tile([65, 512], F32, name="pc", tag="acc")
                    for g in range(4):
                        sp = sp_pool.tile([P, 2048], F32, name="sp", tag="sp")
                        for j in range(4):
                            kb = 4 * g + j
                            nc.tensor.matmul(
                                sp[:, j * 512:(j + 1) * 512],
                                kt[hp][hr:hr + 64, kb * P:(kb + 1) * P],
                                qt[hp][hr:hr + 64, c * 512:(c + 1) * 512],
                                start=True, stop=True)
                        es = es_pool.tile([P, 2048], F32R, name="es", tag="es")
                        nc.scalar.activation(es[:], sp[:], AF.Exp)
                        for j in range(4):
                            kb = 4 * g + j
                            nc.tensor.matmul(
                                pc[:], vt[kb][:, h, :],
                                es[:, j * 512:(j + 1) * 512],
                                start=(g == 0 and j == 0),
                                stop=(g == 3 and j == 3))
                    rd = rd_pool.tile([1, 512], F32R, name="rd", tag="rd")
                    with nc.allow_low_precision("f32r feeds PE broadcast"):
                        nc.vector.reciprocal(rd[:], pc[64:65, :])
                    bc = bc_pool.tile([64, 512], F32, name="bc", tag="bc")
                    nc.tensor.matmul(bc[:], ones64[:], rd[:],
                                     start=True, stop=True)
                    bcs = bcs_pool.tile([64, 512], F32, name="bcs", tag="bcs")
                    nc.any.tensor_copy(bcs[:], bc[:])
                    nc.vector.tensor_mul(cx[hp][hr:hr + 64, :],
                                         pc[0:64, :], bcs[:])

                # output projection for this query chunk
                for ob in range(H // P):
                    po = acc_pool.tile([P, 512], F32, name="po", tag="acc")
                    for k2 in range(FB):
                        nc.tensor.matmul(
                            po[:], wo[k2][:, ob * P:(ob + 1) * P],
                            cx[k2][:], start=(k2 == 0), stop=(k2 == FB - 1))
                    so = outst.tile([P, 512], F32, name="so", tag="so")
                    nc.any.tensor_copy(so[:], po[:])
                    nc.sync.dma_start(
                        io["outT"][ob * P:(ob + 1) * P,
                                   c * 512:(c + 1) * 512], so[:])


def _build():
    nc = bacc.Bacc("TRN2", target_bir_lowering=False, debug=False,
                   enable_asserts=False)
    io = {
        "xT": nc.dram_tensor("xT", (H, L), F32, kind="ExternalInput").ap(),
        "wq": nc.dram_tensor("wq", (H, HH), F32, kind="ExternalInput").ap(),
        "wk": nc.dram_tensor("wk", (H, HH), F32, kind="ExternalInput").ap(),
        "wv": nc.dram_tensor("wv", (H, HH), F32, kind="ExternalInput").ap(),
        "wo": nc.dram_tensor("wo", (HH, H), F32, kind="ExternalInput").ap(),
        "bq": nc.dram_tensor("bq", (P, FB), F32, kind="ExternalInput").ap(),
        "bk": nc.dram_tensor("bk", (P, FB), F32, kind="ExternalInput").ap(),
        "outT": nc.dram_tensor("outT", (H, L), F32, kind="ExternalOutput").ap(),
    }
    with tile.TileContext(nc) as tc:
        _emit(nc, tc, io)
    nc.compile()
    return nc


def kernel(x, Wq, bq, Wk, bk, Wv, bv, Wo, bo):
    global LAST_RESULTS
    x = np.asarray(x, dtype=np.float32)
    Wq, bq = np.asarray(Wq, np.float32), np.asarray(bq, np.float32)
    Wk, bk = np.asarray(Wk, np.float32), np.asarray(bk, np.float32)
    Wv, bv = np.asarray(Wv, np.float32), np.asarray(bv, np.float32)
    Wo, bo = np.asarray(Wo, np.float32), np.asarray(bo, np.float32)

    if "nc" not in _CACHE:
        _CACHE["nc"] = _build()
    nc = _CACHE["nc"]

    xTs = [np.ascontiguousarray(x[n].T) for n in range(N)]
    in_maps = []
    for core in range(NCORES):
        n, hg = core // 2, core % 2
        sl = slice(hg * HH, (hg + 1) * HH)
        in_maps.append({
            "xT": xTs[n],
            "wq": np.ascontiguousarray(Wq[sl, :].T),
            "wk": np.ascontiguousarray(Wk[sl, :].T),
            "wv": np.ascontiguousarray(Wv[sl, :].T),
            "wo": np.ascontiguousarray(Wo[:, sl].T),
            "bq": np.ascontiguousarray(bq[sl].reshape(FB, P).T),
            "bk": np.ascontiguousarray(bk[sl].reshape(FB, P).T),
        })

    trace = bool(os.environ.get("KERNEL_TRACE"))
    res = bass_utils.run_bass_kernel_spmd(
        nc, in_maps, core_ids=list(range(NCORES)), trace=trace)
    LAST_RESULTS = res

    const_row = bv @ Wo.T + bo  # softmax rows sum to 1: bv folds to a row
    out = np.empty((N, L, H), dtype=np.float32)
    for n in range(N):
        part = res.results[2 * n]["outT"] + res.results[2 * n + 1]["outT"]
        out[n] = part.T + const_row[None, :]
    return out


# revision 11
# speedup vs baseline: 1.6791x; 1.6791x over previous
"""Trainium2 Bass kernel for a 16-head self-attention encoder block.

Problem (fp32):
    x: (4, 2048, 1024);  Wq/Wk/Wv/Wo: (1024, 1024);  b*: (1024,)
    q/k/v = x @ W*.T + b*   (reshaped to 16 heads x 64)
    out   = softmax(q k^T) v @ Wo.T + bo     (no scaling, no mask)

Sharding over 8 cores: (batch n in 0..3) x (head-group hg in 0..1, 8 heads
each).  Each core computes, for its batch element and its 8 heads:
    QT/KT = (x @ Wq_s.T).T  in [feat(512), seq(2048)] layout
    V     =  x @ Wv_s.T     in [seq(2048), feat(512)] layout (+ ones col)
    per head: esT = exp(K_h Q_h^T) in [k, q] layout (flash-style, no HBM
    round-trip); ctxT_h = V_aug_h^T @ esT gives both the unnormalised
    context and the softmax denominators (ones column) in one accumulation;
    normalise, then outT_part = Wo_s^T-slice @ ctxT.
Host side: out[n] = (outT_part[n,0] + outT_part[n,1]).T + (bv @ Wo.T + bo).
The bv/bo terms fold into a constant row because softmax rows sum to 1.

Matmuls run in float32r (TF32-like, full PE rate for moving dim >= 256).
"""
import os
import numpy as np

import concourse.bacc as bacc
import concourse.tile as tile
from concourse import mybir, bass_utils

F32 = mybir.dt.float32
F32R = mybir.dt.float32r
BF16 = mybir.dt.bfloat16
AF = mybir.ActivationFunctionType

N, L, H = 4, 2048, 1024
HH = H // 2          # per-core head dim (8 heads x 64)
P = 128
KC = H // P          # 8 contraction chunks for QKV projections
FB = HH // P         # 4 feature pblocks per projection
QC = L // 512        # 4 query chunks
KB = L // P          # 16 key blocks
HPC = 8              # heads per core
NCORES = 8

_CACHE: dict = {}
LAST_RESULTS = None


def _emit(nc, tc, io):
    from contextlib import ExitStack

    with ExitStack() as ctx:
        persist = ctx.enter_context(tc.tile_pool(name="persist", bufs=1))

        # --- long-lived weight / bias / constant tiles ---
        w_tiles = {}
        for wname in ("wq", "wk", "wv"):
            for kc in range(KC):
                t = persist.tile([P, HH], F32R, name=f"{wname}{kc}",
                                 tag=f"{wname}{kc}")
                nc.sync.dma_start(t[:], io[wname][kc * P:(kc + 1) * P, :].bitcast(F32R))
                w_tiles[(wname, kc)] = t
        bq_sb = persist.tile([P, FB], F32, name="bq_sb", tag="bq_sb")
        nc.sync.dma_start(bq_sb[:], io["bq"][:])
        bk_sb = persist.tile([P, FB], F32, name="bk_sb", tag="bk_sb")
        nc.sync.dma_start(bk_sb[:], io["bk"][:])
        ones8 = persist.tile([P, HPC], F32, name="ones8", tag="ones8")
        nc.vector.memset(ones8[:], 1.0)
        ones64f = persist.tile([1, 64], F32, name="ones64f", tag="ones64f")
        nc.vector.memset(ones64f[:], 1.0)

        qt = [persist.tile([P, L], F32R, name=f"qt{i}", tag=f"qt{i}")
              for i in range(FB)]
        kt = [persist.tile([P, L], F32R, name=f"kt{i}", tag=f"kt{i}")
              for i in range(FB)]
        vt = [persist.tile([P, HPC, 65], BF16, name=f"v{sb}", tag=f"v{sb}")
              for sb in range(KB)]

        # ================= phase 1: QKV projections =================
        with tc.tile_pool(name="xt", bufs=8) as xt_pool, \
             tc.tile_pool(name="ppj", bufs=4, space="PSUM") as ppj:
            for ch in range(2):            # column halves of x^T (seq dim)
                xts = []
                for kc in range(KC):
                    t = xt_pool.tile([P, L // 2], F32R, name=f"xt{ch}_{kc}",
                                     tag="xt")
                    nc.sync.dma_start(
                        t[:],
                        io["xT"][kc * P:(kc + 1) * P,
                                 ch * (L // 2):(ch + 1) * (L // 2)].bitcast(F32R))
                    xts.append(t)

                # QT / KT: [feat, seq] = W_s @ x^T
                for wname, dst, bias in (("wq", qt, bq_sb), ("wk", kt, bk_sb)):
                    for f in range(FB):
                        for c in range(2):   # 512-wide seq chunks in half
                            pj = ppj.tile([P, 512], F32, name="pj", tag="pj")
                            for kc in range(KC):
                                nc.tensor.matmul(
                                    pj[:],
                                    w_tiles[(wname, kc)][:, f * P:(f + 1) * P],
                                    xts[kc][:, c * 512:(c + 1) * 512],
                                    start=(kc == 0), stop=(kc == KC - 1))
                            col0 = ch * (L // 2) + c * 512
                            nc.scalar.add(dst[f][:, col0:col0 + 512], pj[:],
                                          bias[:, f:f + 1])
                # V: [seq, feat] = x @ Wv_s.T  (+ ones column per head)
                for s in range(KB // 2):
                    sb = ch * (KB // 2) + s
                    pj = ppj.tile([P, 512], F32, name="pj", tag="pj")
                    for kc in range(KC):
                        nc.tensor.matmul(
                            pj[:],
                            xts[kc][:, s * P:(s + 1) * P],
                            w_tiles[("wv", kc)][:],
                            start=(kc == 0), stop=(kc == KC - 1))
                    nc.vector.tensor_copy(
                        vt[sb][:, :, 0:64],
                        pj[:].rearrange("p (h s) -> p h s", h=HPC))
                    nc.vector.tensor_copy(vt[sb][:, :, 64], ones8[:])

        # ================= phase 2: attention + output projection =========
        with ExitStack() as ctx2:
            p2 = ctx2.enter_context(tc.tile_pool(name="p2", bufs=1))
            es_pool = ctx2.enter_context(tc.tile_pool(name="es", bufs=4))
            rd_pool = ctx2.enter_context(tc.tile_pool(name="rd", bufs=2))
            bcs_pool = ctx2.enter_context(tc.tile_pool(name="bcs", bufs=2))
            outst = ctx2.enter_context(tc.tile_pool(name="outst", bufs=4))
            sp_pool = ctx2.enter_context(
                tc.tile_pool(name="sp", bufs=2, space="PSUM"))
            acc_pool = ctx2.enter_context(
                tc.tile_pool(name="acc", bufs=4, space="PSUM"))

            wo = []
            for k2 in range(FB):
                t = p2.tile([P, H], F32R, name=f"wo{k2}", tag=f"wo{k2}")
                nc.sync.dma_start(
                    t[:], io["wo"][k2 * P:(k2 + 1) * P, :].bitcast(F32R))
                wo.append(t)
            ctx_pool = ctx2.enter_context(tc.tile_pool(name="cx", bufs=2))

            for c in range(QC):
                cx = [ctx_pool.tile([P, 512], F32R, name=f"cx{i}",
                                    tag=f"cx{i}") for i in range(FB)]
                for p in range(HPC // 2):   # head pairs at partitions 0/64
                    h0, h1 = 2 * p, 2 * p + 1
                    pc0 = acc_pool.tile([65, 512], F32, name="pc0", tag="acc")
                    pc1 = acc_pool.tile([65, 512], F32, name="pc1", tag="acc")
                    for kb in range(KB):
                        # scores for both heads land on disjoint PE row
                        # groups (T0/T8) and separate PSUM banks
                        sp = sp_pool.tile([P, 1024], F32, name="sp", tag="sp")
                        nc.tensor.matmul(
                            sp[:, 0:512],
                            kt[p][0:64, kb * P:(kb + 1) * P],
                            qt[p][0:64, c * 512:(c + 1) * 512],
                            start=True, stop=True)
                        nc.tensor.matmul(
                            sp[:, 512:1024],
                            kt[p][64:128, kb * P:(kb + 1) * P],
                            qt[p][64:128, c * 512:(c + 1) * 512],
                            start=True, stop=True)
                        es = es_pool.tile([P, 1024], BF16, name="es", tag="es")
                        nc.scalar.activation(es[:], sp[:], AF.Exp)
                        nc.tensor.matmul(pc0[:], vt[kb][:, h0, :],
                                         es[:, 0:512],
                                         start=(kb == 0), stop=(kb == KB - 1))
                        nc.tensor.matmul(pc1[:], vt[kb][:, h1, :],
                                         es[:, 512:1024],
                                         start=(kb == 0), stop=(kb == KB - 1))
                    for hh, pc in ((h0, pc0), (h1, pc1)):
                        hr = (hh % 2) * 64
                        dsb = rd_pool.tile([1, 512], F32, name="dsb",
                                           tag="dsb")
                        nc.vector.tensor_copy(dsb[:], pc[64:65, :])
                        rd = rd_pool.tile([1, 512], F32, name="rd", tag="rd")
                        nc.vector.reciprocal_approx_fast(rd[:], dsb[:])
                        bc = acc_pool.tile([64, 512], F32, name="bc",
                                           tag="acc")
                        nc.tensor.matmul(bc[:], ones64f[:], rd[:],
                                         start=True, stop=True)
                        bcs = bcs_pool.tile([64, 512], F32, name="bcs",
                                            tag="bcs")
                        nc.any.tensor_copy(bcs[:], bc[:])
                        nc.vector.tensor_mul(cx[p][hr:hr + 64, :],
                                             pc[0:64, :], bcs[:])

                # output projection for this query chunk
                for ob in range(H // P):
                    po = acc_pool.tile([P, 512], F32, name="po", tag="acc")
                    for k2 in range(FB):
                        nc.tensor.matmul(
                            po[:], wo[k2][:, ob * P:(ob + 1) * P],
                            cx[k2][:], start=(k2 == 0), stop=(k2 == FB - 1))
                    so = outst.tile([P, 512], F32, name="so", tag="so")
                    nc.any.tensor_copy(so[:], po[:])
                    nc.sync.dma_start(
                        io["outT"][ob * P:(ob + 1) * P,
                                   c * 512:(c + 1) * 512], so[:])


def _build():
    nc = bacc.Bacc("TRN2", target_bir_lowering=False, debug=False,
                   enable_asserts=False)
    io = {
        "xT": nc.dram_tensor("xT", (H, L), F32, kind="ExternalInput").ap(),
        "wq": nc.dram_tensor("wq", (H, HH), F32, kind="ExternalInput").ap(),
        "wk": nc.dram_tensor("wk", (H, HH), F32, kind="ExternalInput").ap(),
        "wv": nc.dram_tensor("wv", (H, HH), F32, kind="ExternalInput").ap(),
        "wo": nc.dram_tensor("wo", (HH, H), F32, kind="ExternalInput").ap(),
        "bq": nc.dram_tensor("bq", (P, FB), F32, kind="ExternalInput").ap(),
        "bk": nc.dram_tensor("bk", (P, FB), F32, kind="ExternalInput").ap(),
        "outT": nc.dram_tensor("outT", (H, L), F32, kind="ExternalOutput").ap(),
    }
    with tile.TileContext(nc) as tc:
        _emit(nc, tc, io)
    nc.compile()
    return nc


def kernel(x, Wq, bq, Wk, bk, Wv, bv, Wo, bo):
    global LAST_RESULTS
    x = np.asarray(x, dtype=np.float32)
    Wq, bq = np.asarray(Wq, np.float32), np.asarray(bq, np.float32)
    Wk, bk = np.asarray(Wk, np.float32), np.asarray(bk, np.float32)
    Wv, bv = np.asarray(Wv, np.float32), np.asarray(bv, np.float32)
    Wo, bo = np.asarray(Wo, np.float32), np.asarray(bo, np.float32)

    if "nc" not in _CACHE:
        _CACHE["nc"] = _build()
    nc = _CACHE["nc"]

    xTs = [np.ascontiguousarray(x[n].T) for n in range(N)]
    in_maps = []
    for core in range(NCORES):
        n, hg = core // 2, core % 2
        sl = slice(hg * HH, (hg + 1) * HH)
        in_maps.append({
            "xT": xTs[n],
            "wq": np.ascontiguousarray(Wq[sl, :].T),
            "wk": np.ascontiguousarray(Wk[sl, :].T),
            "wv": np.ascontiguousarray(Wv[sl, :].T),
            "wo": np.ascontiguousarray(Wo[:, sl].T),
            "bq": np.ascontiguousarray(bq[sl].reshape(FB, P).T),
            "bk": np.ascontiguousarray(bk[sl].reshape(FB, P).T),
        })

    trace = bool(os.environ.get("KERNEL_TRACE"))
    res = bass_utils.run_bass_kernel_spmd(
        nc, in_maps, core_ids=list(range(NCORES)), trace=trace)
    LAST_RESULTS = res

    const_row = bv @ Wo.T + bo  # softmax rows sum to 1: bv folds to a row
    out = np.empty((N, L, H), dtype=np.float32)
    for n in range(N):
        part = res.results[2 * n]["outT"] + res.results[2 * n + 1]["outT"]
        out[n] = part.T + const_row[None, :]
    return out


# revision 17
# speedup vs baseline: 1.6807x; 1.0010x over previous
"""Trainium2 Bass kernel for a 16-head self-attention encoder block.

Problem (fp32):
    x: (4, 2048, 1024);  Wq/Wk/Wv/Wo: (1024, 1024);  b*: (1024,)
    q/k/v = x @ W*.T + b*   (reshaped to 16 heads x 64)
    out   = softmax(q k^T) v @ Wo.T + bo     (no scaling, no mask)

Sharding over 8 cores: (batch n in 0..3) x (head-group hg in 0..1, 8 heads
each).  Each core computes, for its batch element and its 8 heads:
    QT/KT = (x @ Wq_s.T).T  in [feat(512), seq(2048)] layout
    V     =  x @ Wv_s.T     in [seq(2048), feat(512)] layout (+ ones col)
    per head: esT = exp(K_h Q_h^T) in [k, q] layout (flash-style, no HBM
    round-trip); ctxT_h = V_aug_h^T @ esT gives both the unnormalised
    context and the softmax denominators (ones column) in one accumulation;
    normalise, then outT_part = Wo_s^T-slice @ ctxT.
Host side: out[n] = (outT_part[n,0] + outT_part[n,1]).T + (bv @ Wo.T + bo).
The bv/bo terms fold into a constant row because softmax rows sum to 1.

Matmuls run in float32r (TF32-like, full PE rate for moving dim >= 256).
"""
import os
import numpy as np

import concourse.bacc as bacc
import concourse.tile as tile
from concourse import mybir, bass_utils

F32 = mybir.dt.float32
F32R = mybir.dt.float32r
BF16 = mybir.dt.bfloat16
AF = mybir.ActivationFunctionType

N, L, H = 4, 2048, 1024
HH = H // 2          # per-core head dim (8 heads x 64)
P = 128
KC = H // P          # 8 contraction chunks for QKV projections
FB = HH // P         # 4 feature pblocks per projection
QC = L // 512        # 4 query chunks
KB = L // P          # 16 key blocks
HPC = 8              # heads per core
NCORES = 8

_CACHE: dict = {}
LAST_RESULTS = None


def _emit(nc, tc, io):
    from contextlib import ExitStack

    with ExitStack() as ctx:
        persist = ctx.enter_context(tc.tile_pool(name="persist", bufs=1))

        # --- long-lived weight / bias / constant tiles ---
        w_tiles = {}
        for wname in ("wq", "wk", "wv"):
            for kc in range(KC):
                t = persist.tile([P, HH], F32R, name=f"{wname}{kc}",
                                 tag=f"{wname}{kc}")
                nc.sync.dma_start(t[:], io[wname][kc * P:(kc + 1) * P, :].bitcast(F32R))
                w_tiles[(wname, kc)] = t
        bq_sb = persist.tile([P, FB], F32, name="bq_sb", tag="bq_sb")
        nc.sync.dma_start(bq_sb[:], io["bq"][:])
        bk_sb = persist.tile([P, FB], F32, name="bk_sb", tag="bk_sb")
        nc.sync.dma_start(bk_sb[:], io["bk"][:])
        ones8 = persist.tile([P, HPC], F32, name="ones8", tag="ones8")
        nc.vector.memset(ones8[:], 1.0)
        ones64f = persist.tile([1, 64], F32, name="ones64f", tag="ones64f")
        nc.vector.memset(ones64f[:], 1.0)
        ones64 = persist.tile([1, 64], F32R, name="ones64", tag="ones64")
        nc.scalar.copy(ones64[:], ones64f[:])

        qt = [persist.tile([P, L], F32R, name=f"qt{i}", tag=f"qt{i}")
              for i in range(FB)]
        kt = [persist.tile([P, L], F32R, name=f"kt{i}", tag=f"kt{i}")
              for i in range(FB)]
        vt = [persist.tile([P, HPC, 65], BF16, name=f"v{sb}", tag=f"v{sb}")
              for sb in range(KB)]

        # ================= phase 1: QKV projections =================
        with tc.tile_pool(name="xt", bufs=8) as xt_pool, \
             tc.tile_pool(name="ppj", bufs=4, space="PSUM") as ppj:
            for ch in range(2):            # column halves of x^T (seq dim)
                xts = []
                for kc in range(KC):
                    t = xt_pool.tile([P, L // 2], F32R, name=f"xt{ch}_{kc}",
                                     tag="xt")
                    nc.sync.dma_start(
                        t[:],
                        io["xT"][kc * P:(kc + 1) * P,
                                 ch * (L // 2):(ch + 1) * (L // 2)].bitcast(F32R))
                    xts.append(t)

                # QT / KT: [feat, seq] = W_s @ x^T
                for wname, dst, bias in (("wq", qt, bq_sb), ("wk", kt, bk_sb)):
                    for f in range(FB):
                        for c in range(2):   # 512-wide seq chunks in half
                            pj = ppj.tile([P, 512], F32, name="pj", tag="pj")
                            for kc in range(KC):
                                nc.tensor.matmul(
                                    pj[:],
                                    w_tiles[(wname, kc)][:, f * P:(f + 1) * P],
                                    xts[kc][:, c * 512:(c + 1) * 512],
                                    start=(kc == 0), stop=(kc == KC - 1))
                            col0 = ch * (L // 2) + c * 512
                            nc.vector.tensor_scalar_add(
                                dst[f][:, col0:col0 + 512], pj[:],
                                bias[:, f:f + 1])
                # V: [seq, feat] = x @ Wv_s.T  (+ ones column per head)
                for s in range(KB // 2):
                    sb = ch * (KB // 2) + s
                    pj = ppj.tile([P, 512], F32, name="pj", tag="pj")
                    for kc in range(KC):
                        nc.tensor.matmul(
                            pj[:],
                            xts[kc][:, s * P:(s + 1) * P],
                            w_tiles[("wv", kc)][:],
                            start=(kc == 0), stop=(kc == KC - 1))
                    nc.vector.tensor_copy(
                        vt[sb][:, :, 0:64],
                        pj[:].rearrange("p (h s) -> p h s", h=HPC))
                    nc.vector.tensor_copy(vt[sb][:, :, 64], ones8[:])

        # ================= phase 2: attention + output projection =========
        with ExitStack() as ctx2:
            p2 = ctx2.enter_context(tc.tile_pool(name="p2", bufs=1))
            es_pool = ctx2.enter_context(tc.tile_pool(name="es", bufs=4))
            rd_pool = ctx2.enter_context(tc.tile_pool(name="rd", bufs=2))
            bcs_pool = ctx2.enter_context(tc.tile_pool(name="bcs", bufs=2))
            dcp_pool = ctx2.enter_context(tc.tile_pool(name="dcp", bufs=4))
            outst = ctx2.enter_context(tc.tile_pool(name="outst", bufs=4))
            sp_pool = ctx2.enter_context(
                tc.tile_pool(name="sp", bufs=2, space="PSUM"))
            acc_pool = ctx2.enter_context(
                tc.tile_pool(name="acc", bufs=4, space="PSUM"))

            wo = []
            for k2 in range(FB):
                t = p2.tile([P, H], F32R, name=f"wo{k2}", tag=f"wo{k2}")
                nc.sync.dma_start(
                    t[:], io["wo"][k2 * P:(k2 + 1) * P, :].bitcast(F32R))
                wo.append(t)
            ctx_pool = ctx2.enter_context(tc.tile_pool(name="cx", bufs=2))

            for c in range(QC):
                cx = [ctx_pool.tile([P, 512], F32R, name=f"cx{i}",
                                    tag=f"cx{i}") for i in range(FB)]
                for p in range(HPC // 2):   # head pairs at partitions 0/64
                    h0, h1 = 2 * p, 2 * p + 1
                    pc0 = acc_pool.tile([65, 512], F32, name="pc0", tag="acc")
                    pc1 = acc_pool.tile([65, 512], F32, name="pc1", tag="acc")
                    for kb in range(KB):
                        # scores for both heads land on disjoint PE row
                        # groups (T0/T8) and separate PSUM banks
                        sp = sp_pool.tile([P, 1024], F32, name="sp", tag="sp")
                        nc.tensor.matmul(
                            sp[:, 0:512],
                            kt[p][0:64, kb * P:(kb + 1) * P],
                            qt[p][0:64, c * 512:(c + 1) * 512],
                            start=True, stop=True)
                        nc.tensor.matmul(
                            sp[:, 512:1024],
                            kt[p][64:128, kb * P:(kb + 1) * P],
                            qt[p][64:128, c * 512:(c + 1) * 512],
                            start=True, stop=True)
                        es = es_pool.tile([P, 1024], BF16, name="es", tag="es")
                        nc.scalar.activation(es[:], sp[:], AF.Exp)
                        nc.tensor.matmul(pc0[:], vt[kb][:, h0, :],
                                         es[:, 0:512],
                                         start=(kb == 0), stop=(kb == KB - 1))
                        nc.tensor.matmul(pc1[:], vt[kb][:, h1, :],
                                         es[:, 512:1024],
                                         start=(kb == 0), stop=(kb == KB - 1))
                    # evacuate accumulators to SBUF immediately so the next
                    # pair's PSUM slots free up; normalize from the copies
                    dcps = []
                    for pc in (pc0, pc1):
                        dcp = dcp_pool.tile([65, 512], F32, name="dcp",
                                            tag="dcp")
                        nc.vector.tensor_copy(dcp[:], pc[:])
                        dcps.append(dcp)
                    for hh, dcp in ((h0, dcps[0]), (h1, dcps[1])):
                        hr = (hh % 2) * 64
                        # custom-DVE recip needs a base-partition-0 SBUF input
                        dsb = rd_pool.tile([1, 512], F32, name="dsb",
                                           tag="dsb")
                        nc.vector.tensor_copy(dsb[:], dcp[64:65, :])
                        rd = rd_pool.tile([1, 512], F32, name="rd", tag="rd")
                        nc.vector.reciprocal_approx_fast(rd[:], dsb[:])
                        rdr = rd_pool.tile([1, 512], F32R, name="rdr",
                                           tag="rdr")
                        with nc.allow_low_precision("f32r rhs for broadcast"):
                            nc.vector.tensor_copy(rdr[:], rd[:])
                        bc = acc_pool.tile([64, 512], F32, name="bc",
                                           tag="acc")
                        nc.tensor.matmul(bc[:], ones64[:], rdr[:],
                                         start=True, stop=True)
                        bcs = bcs_pool.tile([64, 512], F32, name="bcs",
                                            tag="bcs")
                        nc.vector.tensor_copy(bcs[:], bc[:])
                        nc.vector.tensor_mul(cx[p][hr:hr + 64, :],
                                             dcp[0:64, :], bcs[:])

                # output projection for this query chunk
                for ob in range(H // P):
                    po = acc_pool.tile([P, 512], F32, name="po", tag="acc")
                    for k2 in range(FB):
                        nc.tensor.matmul(
                            po[:], wo[k2][:, ob * P:(ob + 1) * P],
                            cx[k2][:], start=(k2 == 0), stop=(k2 == FB - 1))
                    so = outst.tile([P, 512], F32, name="so", tag="so")
                    nc.vector.tensor_copy(so[:], po[:])
                    nc.sync.dma_start(
                        io["outT"][ob * P:(ob + 1) * P,
                                   c * 512:(c + 1) * 512], so[:])


def _build():
    nc = bacc.Bacc("TRN2", target_bir_lowering=False, debug=False,
                   enable_asserts=False)
    io = {
        "xT": nc.dram_tensor("xT", (H, L), F32, kind="ExternalInput").ap(),
        "wq": nc.dram_tensor("wq", (H, HH), F32, kind="ExternalInput").ap(),
        "wk": nc.dram_tensor("wk", (H, HH), F32, kind="ExternalInput").ap(),
        "wv": nc.dram_tensor("wv", (H, HH), F32, kind="ExternalInput").ap(),
        "wo": nc.dram_tensor("wo", (HH, H), F32, kind="ExternalInput").ap(),
        "bq": nc.dram_tensor("bq", (P, FB), F32, kind="ExternalInput").ap(),
        "bk": nc.dram_tensor("bk", (P, FB), F32, kind="ExternalInput").ap(),
        "outT": nc.dram_tensor("outT", (H, L), F32, kind="ExternalOutput").ap(),
    }
    with tile.TileContext(nc) as tc:
        _emit(nc, tc, io)
    nc.compile()
    return nc


def kernel(x, Wq, bq, Wk, bk, Wv, bv, Wo, bo):
    global LAST_RESULTS
    x = np.asarray(x, dtype=np.float32)
    Wq, bq = np.asarray(Wq, np.float32), np.asarray(bq, np.float32)
    Wk, bk = np.asarray(Wk, np.float32), np.asarray(bk, np.float32)
    Wv, bv = np.asarray(Wv, np.float32), np.asarray(bv, np.float32)
    Wo, bo = np.asarray(Wo, np.float32), np.asarray(bo, np.float32)

    if "nc" not in _CACHE:
        _CACHE["nc"] = _build()
    nc = _CACHE["nc"]

    xTs = [np.ascontiguousarray(x[n].T) for n in range(N)]
    in_maps = []
    for core in range(NCORES):
        n, hg = core // 2, core % 2
        sl = slice(hg * HH, (hg + 1) * HH)
        in_maps.append({
            "xT": xTs[n],
            "wq": np.ascontiguousarray(Wq[sl, :].T),
            "wk": np.ascontiguousarray(Wk[sl, :].T),
            "wv": np.ascontiguousarray(Wv[sl, :].T),
            "wo": np.ascontiguousarray(Wo[:, sl].T),
            "bq": np.ascontiguousarray(bq[sl].reshape(FB, P).T),
            "bk": np.ascontiguousarray(bk[sl].reshape(FB, P).T),
        })

    trace = bool(os.environ.get("KERNEL_TRACE"))
    res = bass_utils.run_bass_kernel_spmd(
        nc, in_maps, core_ids=list(range(NCORES)), trace=trace)
    LAST_RESULTS = res

    const_row = bv @ Wo.T + bo  # softmax rows sum to 1: bv folds to a row
    out = np.empty((N, L, H), dtype=np.float32)
    for n in range(N):
        part = res.results[2 * n]["outT"] + res.results[2 * n + 1]["outT"]
        out[n] = part.T + const_row[None, :]
    return out


# revision 18
# speedup vs baseline: 1.9535x; 1.1623x over previous
"""Trainium2 Bass kernel for a 16-head self-attention encoder block.

Problem (fp32):
    x: (4, 2048, 1024);  Wq/Wk/Wv/Wo: (1024, 1024);  b*: (1024,)
    q/k/v = x @ W*.T + b*   (reshaped to 16 heads x 64)
    out   = softmax(q k^T) v @ Wo.T + bo     (no scaling, no mask)

Sharding over 8 cores: (batch n in 0..3) x (head-group hg in 0..1, 8 heads
each).  Each core computes, for its batch element and its 8 heads:
    QT/KT = (x @ W_s.T).T  in [feat, seq] layout;  KT is stored per-head
    zero-padded to 128 partitions so score matmuls contract K=128 and the
    whole kernel stays in the PE's 128-row tiling mode (no mode-switch
    drains).  V = x @ Wv_s.T in [seq, feat] layout with a ones column per
    head: the ctx matmul then yields the softmax denominators for free.
    esT = exp(K_h Q_h^T) in [k, q] layout (flash-style, stays on-chip);
    ctxT_h = V_aug_h^T @ esT; normalise via reciprocal_approx_fast and a
    PE broadcast; outT_part = Wo_s^T-slice @ ctxT.
Host side: out[n] = (outT_part[n,0] + outT_part[n,1]).T + (bv @ Wo.T + bo)
(bias-of-V folds into a constant row because softmax rows sum to 1).

Projections/scores/output-projection run in float32r (TF32-like); the
attention-weight matmul runs in bf16 (es and V), which measured ~2e-3
scaled max error overall.
"""
import os
import numpy as np

import concourse.bacc as bacc
import concourse.tile as tile
from concourse import mybir, bass_utils

F32 = mybir.dt.float32
F32R = mybir.dt.float32r
BF16 = mybir.dt.bfloat16
AF = mybir.ActivationFunctionType

N, L, H = 4, 2048, 1024
HH = H // 2          # per-core head dim (8 heads x 64)
P = 128
KC = H // P          # 8 contraction chunks for QKV projections
FB = HH // P         # 4 feature pblocks per projection
QC = L // 512        # 4 query chunks
KB = L // P          # 16 key blocks
HPC = 8              # heads per core
NCORES = 8

_CACHE: dict = {}
LAST_RESULTS = None


def _emit(nc, tc, io):
    from contextlib import ExitStack

    with ExitStack() as ctx:
        persist = ctx.enter_context(tc.tile_pool(name="persist", bufs=1))

        # --- long-lived tiles ---
        bq_sb = persist.tile([P, FB], F32, name="bq_sb", tag="bq_sb")
        nc.sync.dma_start(bq_sb[:], io["bq"][:])
        bk_sb = persist.tile([P, FB], F32, name="bk_sb", tag="bk_sb")
        nc.sync.dma_start(bk_sb[:], io["bk"][:])
        ones8 = persist.tile([P, HPC], F32, name="ones8", tag="ones8")
        nc.vector.memset(ones8[:], 1.0)
        ones64f = persist.tile([1, 64], F32, name="ones64f", tag="ones64f")
        nc.vector.memset(ones64f[:], 1.0)
        ones64 = persist.tile([1, 64], F32R, name="ones64", tag="ones64")
        nc.scalar.copy(ones64[:], ones64f[:])

        qt = [persist.tile([P, L], F32R, name=f"qt{i}", tag=f"qt{i}")
              for i in range(FB)]
        # per-head KT, zero-padded to K=128: head h's 64 dims sit at their
        # natural parity rows; the other 64 rows are zero
        ktz = [persist.tile([P, L], F32R, name=f"ktz{h}", tag=f"ktz{h}")
               for h in range(HPC)]
        vt = [persist.tile([P, HPC, 65], BF16, name=f"v{sb}", tag=f"v{sb}")
              for sb in range(KB)]

        # ================= phase 1: QKV projections =================
        with tc.tile_pool(name="ph1", bufs=1) as ph1, \
             tc.tile_pool(name="xt", bufs=8) as xt_pool, \
             tc.tile_pool(name="ppj", bufs=4, space="PSUM") as ppj:
            w_tiles = {}
            for wname in ("wq", "wk", "wv"):
                for kc in range(KC):
                    t = ph1.tile([P, HH], F32R, name=f"{wname}{kc}",
                                 tag=f"{wname}{kc}")
                    nc.sync.dma_start(
                        t[:], io[wname][kc * P:(kc + 1) * P, :].bitcast(F32R))
                    w_tiles[(wname, kc)] = t
            zsc = ph1.tile([64, L], F32, name="zsc", tag="zsc")
            nc.vector.memset(zsc[:], 0.0)
            for h in range(HPC):
                zr = (1 - h % 2) * 64   # the zero half of head h's KT tile
                nc.scalar.copy(ktz[h][zr:zr + 64, :], zsc[:])

            for ch in range(2):            # column halves of x^T (seq dim)
                xts = []
                for kc in range(KC):
                    t = xt_pool.tile([P, L // 2], F32R, name=f"xt{ch}_{kc}",
                                     tag="xt")
                    nc.sync.dma_start(
                        t[:],
                        io["xT"][kc * P:(kc + 1) * P,
                                 ch * (L // 2):(ch + 1) * (L // 2)].bitcast(F32R))
                    xts.append(t)

                # V: [seq, feat] = x @ Wv_s.T  (+ ones column per head)
                for s in range(KB // 2):
                    sb = ch * (KB // 2) + s
                    pj = ppj.tile([P, 512], F32, name="pj", tag="pj")
                    for kc in range(KC):
                        nc.tensor.matmul(
                            pj[:],
                            xts[kc][:, s * P:(s + 1) * P],
                            w_tiles[("wv", kc)][:],
                            start=(kc == 0), stop=(kc == KC - 1))
                    nc.vector.tensor_copy(
                        vt[sb][:, :, 0:64],
                        pj[:].rearrange("p (h s) -> p h s", h=HPC))
                    nc.vector.tensor_copy(vt[sb][:, :, 64], ones8[:])

                # KT (zero-padded per head) and QT: [feat, seq] = W_s @ x^T
                for wname in ("wk", "wq"):
                    for f in range(FB):
                        for c in range(2):
                            pj = ppj.tile([P, 512], F32, name="pj", tag="pj")
                            for kc in range(KC):
                                nc.tensor.matmul(
                                    pj[:],
                                    w_tiles[(wname, kc)][:, f * P:(f + 1) * P],
                                    xts[kc][:, c * 512:(c + 1) * 512],
                                    start=(kc == 0), stop=(kc == KC - 1))
                            col0 = ch * (L // 2) + c * 512
                            if wname == "wq":
                                nc.vector.tensor_scalar_add(
                                    qt[f][:, col0:col0 + 512], pj[:],
                                    bq_sb[:, f:f + 1])
                            else:
                                for par in range(2):   # split to head tiles
                                    r = par * 64
                                    nc.vector.tensor_scalar_add(
                                        ktz[2 * f + par][r:r + 64,
                                                         col0:col0 + 512],
                                        pj[r:r + 64, :],
                                        bk_sb[r:r + 64, f:f + 1])

        # ================= phase 2: attention + output projection =========
        with ExitStack() as ctx2:
            p2 = ctx2.enter_context(tc.tile_pool(name="p2", bufs=1))
            es_pool = ctx2.enter_context(tc.tile_pool(name="es", bufs=3))
            rd_pool = ctx2.enter_context(tc.tile_pool(name="rd", bufs=2))
            bcs_pool = ctx2.enter_context(tc.tile_pool(name="bcs", bufs=2))
            dcp_pool = ctx2.enter_context(tc.tile_pool(name="dcp", bufs=4))
            outst = ctx2.enter_context(tc.tile_pool(name="outst", bufs=4))
            sp_pool = ctx2.enter_context(
                tc.tile_pool(name="sp", bufs=3, space="PSUM"))
            acc_pool = ctx2.enter_context(
                tc.tile_pool(name="acc", bufs=2, space="PSUM"))

            wo = []
            for k2 in range(FB):
                t = p2.tile([P, H], F32R, name=f"wo{k2}", tag=f"wo{k2}")
                nc.sync.dma_start(
                    t[:], io["wo"][k2 * P:(k2 + 1) * P, :].bitcast(F32R))
                wo.append(t)
            ctx_pool = ctx2.enter_context(tc.tile_pool(name="cx", bufs=2))

            for c in range(QC):
                cx = [ctx_pool.tile([P, 512], F32R, name=f"cx{i}",
                                    tag=f"cx{i}") for i in range(FB)]
                for h in range(HPC):
                    hp = h // 2
                    pc = acc_pool.tile([65, 512], F32, name="pc", tag="acc")
                    for g in range(KB // 2):
                        sp = sp_pool.tile([P, 1024], F32, name="sp", tag="sp")
                        for j in range(2):
                            kb = 2 * g + j
                            nc.tensor.matmul(
                                sp[:, j * 512:(j + 1) * 512],
                                ktz[h][:, kb * P:(kb + 1) * P],
                                qt[hp][:, c * 512:(c + 1) * 512],
                                start=True, stop=True)
                        es = es_pool.tile([P, 1024], BF16, name="es",
                                          tag="es")
                        nc.scalar.activation(es[:], sp[:], AF.Exp)
                        for j in range(2):
                            kb = 2 * g + j
                            nc.tensor.matmul(
                                pc[:], vt[kb][:, h, :],
                                es[:, j * 512:(j + 1) * 512],
                                start=(kb == 0), stop=(kb == KB - 1))
                    # evacuate accumulator so the next head's PSUM frees up
                    dcp = dcp_pool.tile([65, 512], F32, name="dcp", tag="dcp")
                    nc.vector.tensor_copy(dcp[:], pc[:])
                    hr = (h % 2) * 64
                    # custom-DVE recip needs a base-partition-0 SBUF input
                    dsb = rd_pool.tile([1, 512], F32, name="dsb", tag="dsb")
                    nc.vector.tensor_copy(dsb[:], dcp[64:65, :])
                    rd = rd_pool.tile([1, 512], F32, name="rd", tag="rd")
                    nc.vector.reciprocal_approx_fast(rd[:], dsb[:])
                    rdr = rd_pool.tile([1, 512], F32R, name="rdr", tag="rdr")
                    with nc.allow_low_precision("f32r rhs for broadcast"):
                        nc.vector.tensor_copy(rdr[:], rd[:])
                    bc = acc_pool.tile([64, 512], F32, name="bc", tag="acc")
                    nc.tensor.matmul(bc[:], ones64[:], rdr[:],
                                     start=True, stop=True)
                    bcs = bcs_pool.tile([64, 512], F32, name="bcs", tag="bcs")
                    nc.vector.tensor_copy(bcs[:], bc[:])
                    nc.vector.tensor_mul(cx[hp][hr:hr + 64, :],
                                         dcp[0:64, :], bcs[:])

                # output projection for this query chunk
                for ob in range(H // P):
                    po = acc_pool.tile([P, 512], F32, name="po", tag="acc")
                    for k2 in range(FB):
                        nc.tensor.matmul(
                            po[:], wo[k2][:, ob * P:(ob + 1) * P],
                            cx[k2][:], start=(k2 == 0), stop=(k2 == FB - 1))
                    so = outst.tile([P, 512], F32, name="so", tag="so")
                    nc.vector.tensor_copy(so[:], po[:])
                    nc.sync.dma_start(
                        io["outT"][ob * P:(ob + 1) * P,
                                   c * 512:(c + 1) * 512], so[:])


def _build():
    nc = bacc.Bacc("TRN2", target_bir_lowering=False, debug=False,
                   enable_asserts=False)
    io = {
        "xT": nc.dram_tensor("xT", (H, L), F32, kind="ExternalInput").ap(),
        "wq": nc.dram_tensor("wq", (H, HH), F32, kind="ExternalInput").ap(),
        "wk": nc.dram_tensor("wk", (H, HH), F32, kind="ExternalInput").ap(),
        "wv": nc.dram_tensor("wv", (H, HH), F32, kind="ExternalInput").ap(),
        "wo": nc.dram_tensor("wo", (HH, H), F32, kind="ExternalInput").ap(),
        "bq": nc.dram_tensor("bq", (P, FB), F32, kind="ExternalInput").ap(),
        "bk": nc.dram_tensor("bk", (P, FB), F32, kind="ExternalInput").ap(),
        "outT": nc.dram_tensor("outT", (H, L), F32, kind="ExternalOutput").ap(),
    }
    with tile.TileContext(nc) as tc:
        _emit(nc, tc, io)
    nc.compile()
    return nc


def kernel(x, Wq, bq, Wk, bk, Wv, bv, Wo, bo):
    global LAST_RESULTS
    x = np.asarray(x, dtype=np.float32)
    Wq, bq = np.asarray(Wq, np.float32), np.asarray(bq, np.float32)
    Wk, bk = np.asarray(Wk, np.float32), np.asarray(bk, np.float32)
    Wv, bv = np.asarray(Wv, np.float32), np.asarray(bv, np.float32)
    Wo, bo = np.asarray(Wo, np.float32), np.asarray(bo, np.float32)

    if "nc" not in _CACHE:
        _CACHE["nc"] = _build()
    nc = _CACHE["nc"]

    xTs = [np.ascontiguousarray(x[n].T) for n in range(N)]
    in_maps = []
    for core in range(NCORES):
        n, hg = core // 2, core % 2
        sl = slice(hg * HH, (hg + 1) * HH)
        in_maps.append({
            "xT": xTs[n],
            "wq": np.ascontiguousarray(Wq[sl, :].T),
            "wk": np.ascontiguousarray(Wk[sl, :].T),
            "wv": np.ascontiguousarray(Wv[sl, :].T),
            "wo": np.ascontiguousarray(Wo[:, sl].T),
            "bq": np.ascontiguousarray(bq[sl].reshape(FB, P).T),
            "bk": np.ascontiguousarray(bk[sl].reshape(FB, P).T),
        })

    trace = bool(os.environ.get("KERNEL_TRACE"))
    res = bass_utils.run_bass_kernel_spmd(
        nc, in_maps, core_ids=list(range(NCORES)), trace=trace)
    LAST_RESULTS = res

    const_row = bv @ Wo.T + bo  # softmax rows sum to 1: bv folds to a row
    out = np.empty((N, L, H), dtype=np.float32)
    for n in range(N):
        part = res.results[2 * n]["outT"] + res.results[2 * n + 1]["outT"]
        out[n] = part.T + const_row[None, :]
    return out
